# revision 15
# baseline (speedup 1.0000x reference)
"""Causal multi-head attention on 8 Trainium2 NeuronCores.

Problem: x[2,2048,1024] @ W_Q/K/V[1024,1024] -> 16-head causal attention
(d_head=64) -> @ W_O[1024,1024].

Sharding: tensor-parallel over heads. Core i owns heads 2i, 2i+1 — i.e.
columns [128i:128i+128) of W_Q/W_K/W_V and rows [128i:128i+128) of W_O.
Each core computes its partial output [1024, 4096] (transposed layout,
bf16); the host sums the 8 partials in f32 and un-transposes (the
"all-reduce").

v2 (this file): all-bf16 dataflow tuned for PE occupancy.
  - All matmul operands bf16 (1 cyc/row incl. narrow tiles; fast
    LDWEIGHTS so weight loads hide under matmuls), PSUM accumulates f32.
  - Scores for both heads of a (q-tile, k-chunk) land in one 2-bank
    PSUM unit -> ONE ScalarE exp instruction for both heads (halves
    Activation instruction overhead; ScalarE is the phase-2 co-wall).
  - Softmax denominator via a ones-column in the V tile (PV matmul row
    64), normalized with reciprocal_approx_fast + stride-0 partition
    broadcast (the old [1,512] nc.vector.reciprocal was 3.3us each).
  - W_O matmuls + output DMA are spread through the NEXT q-tile's
    attention loop so the PE never idles at tile boundaries and the
    16.8MB->8.4MB output writeback overlaps compute.
  - Input x, all weights, output: bf16 on the wire (halves HBM traffic;
    rel-err gate is 2e-2, measured ~1e-3).
"""

import contextlib

import ml_dtypes
import numpy as np

import concourse.bass as bass
import concourse.tile as tile
from concourse import bacc, mybir
from concourse.bass_utils import run_bass_kernel_spmd
from concourse.masks import make_identity

F32 = mybir.dt.float32
BF16 = mybir.dt.bfloat16
EXP = mybir.ActivationFunctionType.Exp

N_CORES = 8
P = 128
D = 1024          # d_model
B = 2             # batch
S = 2048          # seq len
T = B * S         # total tokens = 4096
TT = 512          # token tile (free dim of matmuls)
NT = T // TT      # 8 token tiles
KD = D // P       # 8 contraction chunks for projections
JB = S // TT      # 4 q-tiles per batch
CB = S // P       # 16 k-chunks per batch
NCH = T // P      # 32 k-chunks total
H_LOC = 2         # heads per core
DH = 64           # head dim


DEBUG_DUMP = False


def _body(tc):
    nc = tc.nc
    xT = nc.dram_tensor("xT", [D, T], BF16, kind="ExternalInput").ap()
    wq = nc.dram_tensor("wq", [D, P], BF16, kind="ExternalInput").ap()
    wk = nc.dram_tensor("wk", [D, P], BF16, kind="ExternalInput").ap()
    wv = nc.dram_tensor("wv", [D, P], BF16, kind="ExternalInput").ap()
    wo = nc.dram_tensor("wo", [P, D], BF16, kind="ExternalInput").ap()
    outT = nc.dram_tensor("outT", [D, T], BF16, kind="ExternalOutput").ap()

    xT_r = xT.rearrange("(o p) n -> p o n", p=P)
    outT_r = outT.rearrange("(o p) n -> p o n", p=P)

    with contextlib.ExitStack() as ctx:
        const = ctx.enter_context(tc.tile_pool(name="const", bufs=1))
        wpool = ctx.enter_context(tc.tile_pool(name="wpool", bufs=1))
        xpool = ctx.enter_context(tc.tile_pool(name="xpool", bufs=2))
        persist = ctx.enter_context(tc.tile_pool(name="persist", bufs=1))
        prp = ctx.enter_context(tc.tile_pool(name="probs", bufs=6))
        stage = ctx.enter_context(tc.tile_pool(name="stage", bufs=2))
        obp = ctx.enter_context(tc.tile_pool(name="obp", bufs=4))
        psum = ctx.enter_context(tc.tile_pool(name="psum", bufs=2, space="PSUM"))

        # --- constants -----------------------------------------------------
        identity = const.tile([P, P], BF16)
        make_identity(nc, identity)

        # mask_band[k, q] = 1.0 if q >= k else 0.0
        mask_band = const.tile([P, P], BF16)
        nc.gpsimd.memset(mask_band[:], 1.0)
        nc.gpsimd.affine_select(
            out=mask_band[:],
            in_=mask_band[:],
            compare_op=mybir.AluOpType.is_ge,
            fill=0.0,
            base=0,
            pattern=[[1, P]],
            channel_multiplier=-1,
        )

        # --- weights (scalar-engine DMA queue; x tiles own the sync queue) -
        wq_sb = wpool.tile([P, KD, P], BF16)
        nc.scalar.dma_start(wq_sb[:], wq.rearrange("(o p) m -> p o m", p=P))
        wk_sb = wpool.tile([P, KD, P], BF16)
        nc.scalar.dma_start(wk_sb[:], wk.rearrange("(o p) m -> p o m", p=P))
        wv_sb = wpool.tile([P, KD, P], BF16)
        nc.scalar.dma_start(wv_sb[:], wv.rearrange("(o p) m -> p o m", p=P))
        wo_sb = wpool.tile([P, D], BF16)
        nc.scalar.dma_start(wo_sb[:], wo)

        # --- persistent activations ---------------------------------------
        qT = persist.tile([P, T], BF16)       # [2h x 64d, tokens]
        kT = persist.tile([P, T], BF16)
        vn = persist.tile([P, NCH, 2 * DH + 2], BF16)  # [tok, chunk, d0|1|d1|1]
        attnT = persist.tile([P, T], BF16)
        nc.gpsimd.memset(vn[:, :, DH], 1.0)
        nc.gpsimd.memset(vn[:, :, 2 * DH + 1], 1.0)

        # --- projections (emitted in blocks, interleaved into attention) --
        # The V transpose for tile t is deferred (lag-1) so its vt copy has
        # a full block of PE work to land behind; flushed before any q-tile
        # loop that consumes that tile's vn chunks.
        pending_tr = []

        def emit_transposes():
            while pending_tr:
                t, vt = pending_tr.pop(0)
                pt = psum.tile([P, 4, P], BF16, tag="sc", name=f"pt_{t}")
                for s_ in range(4):
                    nc.tensor.transpose(pt[:, s_, :], vt[:, bass.ts(s_, P)],
                                        identity)
                # strided copy drops both heads' dims around the ones cols
                dst = vn[:, bass.ts(t, 4), :].rearrange(
                    "p c (h k) -> p c h k", h=2)[:, :, :, 0:DH]
                src = pt[:].rearrange("p c (h k) -> p c h k", h=2)
                nc.vector.tensor_copy(dst, src)

        def emit_proj(t):
            xt = xpool.tile([P, KD, TT], BF16, name=f"xt_{t}")
            if t == 0:
                # split the very first load so matmuls start sooner
                half = KD // 2
                nc.sync.dma_start(xt[:, 0:half, :],
                                  xT_r[:, 0:half, bass.ts(t, TT)])
                nc.sync.dma_start(xt[:, half:, :],
                                  xT_r[:, half:, bass.ts(t, TT)])
            else:
                nc.sync.dma_start(xt[:], xT_r[:, :, bass.ts(t, TT)])
            uqk = psum.tile([P, 2, TT], F32, tag="sc", name=f"uqk_{t}")
            for k, wsb in ((0, wq_sb), (1, wk_sb)):
                for c in range(KD):
                    nc.tensor.matmul(uqk[:, k, :], wsb[:, c, :], xt[:, c, :],
                                     start=(c == 0), stop=(c == KD - 1))
            nc.vector.tensor_copy(qT[:, bass.ts(t, TT)], uqk[:, 0, :])
            nc.vector.tensor_copy(kT[:, bass.ts(t, TT)], uqk[:, 1, :])
            emit_transposes()
            uv = psum.tile([P, 2, TT], F32, tag="sc", name=f"uv_{t}")
            for c in range(KD):
                nc.tensor.matmul(uv[:, 0, :], wv_sb[:, c, :], xt[:, c, :],
                                 start=(c == 0), stop=(c == KD - 1))
            vt = stage.tile([P, TT], BF16, tag="vt", name=f"vt_{t}")
            nc.vector.tensor_copy(vt[:], uv[:, 0, :])
            pending_tr.append((t, vt))

        emit_proj(0)
        emit_proj(4)

        # --- phase 2: causal attention + interleaved output projection ----
        def pv_step(pvs, jj, ncb, jx, cb, pr):
            b = jx // JB
            c = CB * b + cb
            r = cb - 4 * jj
            lo = P * r if r > 0 else 0
            for h in range(H_LOC):
                nc.tensor.matmul(pvs[(jx, h)][:, lo:],
                                 vn[:, c, bass.ds((DH + 1) * h, DH + 1)],
                                 pr[:, h, lo:],
                                 start=(cb == 0), stop=(cb == ncb - 1))

        def emit_normalize(js, pvs):
            for jx in js:
                jsl = bass.ts(jx, TT)
                for h in range(H_LOC):
                    # custom-DVE recip reads garbage from PSUM; stage to SBUF
                    dn = stage.tile([1, TT], F32, tag="dn",
                                    name=f"dn_{jx}_{h}")
                    nc.vector.tensor_copy(dn[:], pvs[(jx, h)][DH:DH + 1, :])
                    rc = stage.tile([1, TT], F32, tag="rc",
                                    name=f"rc_{jx}_{h}")
                    nc.vector.reciprocal_approx_fast(rc[:], dn[:])
                    rb = stage.tile([DH, TT], F32, tag="rb",
                                    name=f"rb_{jx}_{h}")
                    nc.gpsimd.partition_broadcast(rb[:], rc[:])
                    nc.vector.tensor_mul(
                        attnT[bass.ds(DH * h, DH), jsl],
                        pvs[(jx, h)][0:DH, :], rb[:])

        def emit_wo_unit(jx, fi, ob_eng=None):
            jsl = bass.ts(jx, TT)
            u = psum.tile([P, 2, TT], F32, tag="sc", name=f"wo_{jx}_{fi}")
            for k in range(2):
                nc.tensor.matmul(u[:, k, :], wo_sb[:, bass.ts(2 * fi + k, P)],
                                 attnT[:, jsl], start=True, stop=True)
            ob = obp.tile([P, 2, TT], BF16, tag="ob", name=f"ob_{jx}_{fi}")
            if ob_eng == "scalar":
                nc.scalar.copy(ob[:], u[:])
            else:
                nc.vector.tensor_copy(ob[:], u[:])
            nc.sync.dma_start(outT_r[:, 2 * fi:2 * fi + 2, jsl], ob[:])

        # projection block emitted at the end of stream (jj, si):
        proj_sched = {(0, 0): 1, (0, 1): 5, (1, 0): 2, (1, 1): 6,
                      (2, 0): 3, (2, 1): 7}

        wo_work = []  # (jx, fi) W_O units ready to fill PE gaps

        # Each q-tile (jx) is one "stream": scores/exp/pv over its k-chunks
        # with the sc ring giving 2-chunk lookahead, W_O units of the
        # previous stream as PE filler, normalize at stream end overlapped
        # by the next stream / projection block.
        for jj in range(JB):
            ncb = 4 * (jj + 1)
            emit_transposes()
            for si, jx in enumerate((jj, jj + JB)):
                b = jx // JB
                pvs = {}
                for h in range(H_LOC):
                    pvs[(jx, h)] = psum.tile([DH + 1, TT], F32, tag="a",
                                             bufs=4, name=f"pv_{jx}_{h}")
                pend = None
                for cb in range(ncb):
                    r = cb - 4 * jj
                    lo = P * r if r > 0 else 0
                    c = CB * b + cb
                    u = psum.tile([P, 2, TT], F32, tag="sc",
                                  name=f"sc_{jx}_{cb}")
                    for h in range(H_LOC):
                        hp = slice(DH * h, DH * h + DH)
                        nc.tensor.matmul(u[:, h, lo:], kT[hp, bass.ts(c, P)],
                                         qT[hp, bass.ts(jx, TT)][:, lo:],
                                         start=True, stop=True)
                    pr = prp.tile([P, 2, TT], BF16, tag="pr",
                                  name=f"pr_{jx}_{cb}")
                    nc.scalar.activation(pr[:, :, lo:], u[:, :, lo:], EXP,
                                         scale=0.125)
                    if r >= 0:
                        # zero the upper-triangular part of the diagonal
                        # band for both heads in one multi-dim affine pass
                        nc.gpsimd.affine_select(
                            out=pr[:, :, bass.ts(r, P)],
                            in_=pr[:, :, bass.ts(r, P)],
                            compare_op=mybir.AluOpType.is_ge,
                            fill=0.0,
                            base=0,
                            pattern=[[0, 2], [1, P]],
                            channel_multiplier=-1,
                        )
                    if pend is not None:
                        pv_step(pvs, jj, ncb, jx, *pend)
                    pend = (cb, pr)
                    if cb >= 1 and wo_work:
                        emit_wo_unit(*wo_work.pop(0))
                pv_step(pvs, jj, ncb, jx, *pend)
                emit_normalize((jx,), pvs)
                while wo_work:
                    emit_wo_unit(*wo_work.pop(0))
                if jj < JB - 1 or si == 0:
                    wo_work = [(jx, fi) for fi in range(4)]
                if (jj, si) in proj_sched:
                    emit_proj(proj_sched[(jj, si)])

        # --- tail: last stream's output projection -------------------------
        for fi in range(4):
            emit_wo_unit(JB - 1 + JB, fi, ob_eng=("scalar" if fi % 2 else None))

        if DEBUG_DUMP:
            dq = nc.dram_tensor("dbg_qT", [P, T], BF16,
                                kind="ExternalOutput").ap()
            dk = nc.dram_tensor("dbg_kT", [P, T], BF16,
                                kind="ExternalOutput").ap()
            dv = nc.dram_tensor("dbg_vn", [P, NCH, 2 * DH + 2], BF16,
                                kind="ExternalOutput").ap()
            da = nc.dram_tensor("dbg_attnT", [P, T], BF16,
                                kind="ExternalOutput").ap()
            nc.sync.dma_start(dq, qT[:])
            nc.sync.dma_start(dk, kT[:])
            nc.sync.dma_start(dv, vn[:])
            nc.sync.dma_start(da, attnT[:])


_NC_CACHE = None


def _get_nc():
    global _NC_CACHE
    if _NC_CACHE is None:
        nc = bacc.Bacc("TRN2", target_bir_lowering=False, debug=False,
                       num_devices=N_CORES)
        with tile.TileContext(nc) as tc:
            _body(tc)
        nc.compile()
        _NC_CACHE = nc
    return _NC_CACHE


_BF = ml_dtypes.bfloat16


def _in_maps(x, W_Q, W_K, W_V, W_O):
    xT = np.ascontiguousarray(
        np.asarray(x, dtype=np.float32).reshape(T, D).T).astype(_BF)
    W_Q = np.asarray(W_Q, dtype=np.float32)
    W_K = np.asarray(W_K, dtype=np.float32)
    W_V = np.asarray(W_V, dtype=np.float32)
    W_O = np.asarray(W_O, dtype=np.float32)
    maps = []
    for i in range(N_CORES):
        sl = slice(P * i, P * i + P)
        maps.append({
            "xT": xT,
            "wq": np.ascontiguousarray(W_Q[:, sl]).astype(_BF),
            "wk": np.ascontiguousarray(W_K[:, sl]).astype(_BF),
            "wv": np.ascontiguousarray(W_V[:, sl]).astype(_BF),
            "wo": np.ascontiguousarray(W_O[sl, :]).astype(_BF),
        })
    return maps


def _gather(results):
    acc = np.zeros([D, T], np.float32)
    for r in results:
        acc += r["outT"].astype(np.float32)
    return np.ascontiguousarray(acc.T).reshape(B, S, D)


def kernel(x, W_Q, W_K, W_V, W_O):
    nc = _get_nc()
    res = run_bass_kernel_spmd(nc, _in_maps(x, W_Q, W_K, W_V, W_O),
                               core_ids=list(range(N_CORES)))
    return _gather(res.results)


def kernel_profiled(x, W_Q, W_K, W_V, W_O):
    """Like kernel() but with NTFF tracing.

    Returns (output, exec_time_ns, insts) — insts is the annotated
    gauge instruction list for the traced core (or None).
    """
    nc = _get_nc()
    res = run_bass_kernel_spmd(nc, _in_maps(x, W_Q, W_K, W_V, W_O),
                               core_ids=list(range(N_CORES)), trace=True)
    insts = None
    if res.instructions_and_trace is not None:
        insts = res.instructions_and_trace[0]
    return _gather(res.results), res.exec_time_ns, insts
